# revision 34
# baseline (speedup 1.0000x reference)
"""CoAttention kernel for 8 Trainium2 NeuronCores (v3).

Math (per batch b), refactored so the [Lt, Lv] affinity is never materialized,
and with the elementwise adds folded into the low-rank weights:
    wq_q = T @ w_q                    [Lt, K]
    wv_v = I @ w_v                    [Lv, K]
    A1   = T^T @ wq_q                 [E, K]
    B1   = I^T @ wv_v                 [E, K]
    A2   = w_b^T @ A1 + w_v           [E, K]   (h_v = tanh(I @ A2))
    B2   = w_b @ B1 + w_q             [E, K]   (h_q = tanh(T @ B2))
    av   = softmax(h_v @ w_hv); aq = softmax(h_q @ w_hq)
    out  = tanh((av @ I + aq @ T) @ w_s)       [E]

Sharding: data-parallel over batch. B=64 -> 8 batches per core, weights
replicated. No collectives.

v3 changes vs v2 (PE was the bottleneck at 218us busy / 279us total):
  - wq_q / wv_v are computed DIRECTLY in natural [pos, K] layout
    (stationary = Ttr/Itr chunk, moving = w_q/w_v chunk), eliminating the
    13 PE transposes + PSUM drains per batch that v2 used to convert the
    [K, pos] matmul output.
  - h_v/h_q additions folded into A2/B2 (one [E, K] DVE add each instead of
    [K, 1600] adds), so the [K, pos] orientation of wq_q/wv_v is never needed.
  - The per-batch context row (av @ I + aq @ T) flips orientation: stationary
    = natural data chunk [pc, 128e], moving = attention column [pc, 1],
    accumulating [128, 1] e-columns directly in the Scol16 transposed layout.
    9984 -> ~5000 PE cycles per batch and the output-row transpose pass
    disappears.
"""

import numpy as np

import concourse.bass as bass
import concourse.mybir as mybir
import concourse.tile as tile
from concourse import bass_utils
from concourse.masks import make_identity

# problem shape (hardcoded per contract)
B, LT, LV, E, K = 64, 1024, 576, 768, 128
N_CORES = 8
BPC = B // N_CORES  # batches per core
P = 128
EC = E // P            # 6 chunks of E
LTC = LT // P          # 8 chunks of Lt
LV_CH = [128, 128, 128, 128, 64]   # Lv = 576 = 4*128 + 64
LVC = len(LV_CH)

F32 = mybir.dt.float32
F16 = mybir.dt.float16
TANH = mybir.ActivationFunctionType.Tanh
EXP = mybir.ActivationFunctionType.Exp
COPY = mybir.ActivationFunctionType.Copy


def _split_excess_waits(nc, limit=1):
    """walrus encodes at most one sem wait per hardware instruction; hoist
    extras onto same-engine NOPs placed immediately before."""
    for f in nc.m.functions:
        for bb in f.blocks:
            new_insts = []
            for inst in bb.instructions:
                w = inst.sync_info.on_wait if inst.sync_info else None
                if w and len(w) > limit:
                    extra, keep = w[:-limit], w[-limit:]
                    for j, sw in enumerate(extra):
                        new_insts.append(
                            mybir.InstNoOp(
                                name=f"{inst.name}-waitsplit-{j}",
                                engine=inst.engine,
                                ins=[],
                                outs=[],
                                sync_info=mybir.SyncInfo(on_wait=[sw], on_update=[]),
                            )
                        )
                    inst.sync_info.on_wait = keep
                new_insts.append(inst)
            bb.instructions[:] = new_insts


def build_nc(split_drains=True):
    nc = bass.Bass("TRN2", target_bir_lowering=False, debug=False, num_devices=N_CORES)

    # host-prearranged to SBUF layout: [b, partition, chunk*inner] contiguous
    text = nc.dram_tensor("text", [BPC, P, LTC * E], F16, kind="ExternalInput").ap()
    image = nc.dram_tensor("image", [BPC, P, 4 * E], F16, kind="ExternalInput").ap()
    image5 = nc.dram_tensor("image5", [BPC, 64, E], F16, kind="ExternalInput").ap()
    textT = nc.dram_tensor("textT", [BPC, P, EC * LT], F16, kind="ExternalInput").ap()
    imageT = nc.dram_tensor("imageT", [BPC, P, EC * LV], F16, kind="ExternalInput").ap()
    # all weights host-packed to SBUF layout [P, chunk*inner] so each DMA is
    # 128 contiguous per-partition lines (scattered rearrange-APs cost 10x)
    wq_d = nc.dram_tensor("wq", [P, EC * K], F16, kind="ExternalInput").ap()
    wv_d = nc.dram_tensor("wv", [P, EC * K], F16, kind="ExternalInput").ap()
    wb_d = nc.dram_tensor("wb", [P, EC * E], F16, kind="ExternalInput").ap()
    wbT_d = nc.dram_tensor("wbT", [P, EC * E], F16, kind="ExternalInput").ap()
    whvq_d = nc.dram_tensor("whvq", [P, 2], F16, kind="ExternalInput").ap()
    ws_d = nc.dram_tensor("ws", [P, EC * E], F16, kind="ExternalInput").ap()
    out_d = nc.dram_tensor("out", [BPC, E], F32, kind="ExternalOutput").ap()

    with tile.TileContext(nc) as tc:
        with (
            tc.tile_pool(name="const", bufs=1) as const,
            tc.tile_pool(name="work", bufs=1) as work,
            tc.tile_pool(name="psm", bufs=3, space="PSUM") as psm,    # [P,512] f32
            tc.tile_pool(name="pnat", bufs=2, space="PSUM") as pnat,  # [P,768] f32
            tc.tile_pool(name="pcx", bufs=1, space="PSUM") as pcx,    # [P,8] f32
        ):
            # ---- constants / weights (loaded once) ----
            id32 = const.tile([P, P], F32)
            make_identity(nc, id32)

            # load order matters: the sync HWDGE queue drains FIFO, and the
            # first matmuls (S2' of batch 0) need wv + Itr[0] — put those at
            # the very front so PE starts ~15us earlier.
            wv_sb = const.tile([P, EC, K], F16)
            nc.sync.dma_start(wv_sb[:], wv_d)
            wq_sb = const.tile([P, EC, K], F16)
            whvq_sb = const.tile([P, 2], F16)
            # big weights ride the SWDGE ring, but deferred (emitted inside
            # the batch loop) so they don't steal head HBM bandwidth from
            # batch 0's input loads: wbT/wb aren't needed until S56(0) at
            # ~30us, ws not until the final matmul
            wb_sb = const.tile([P, EC, E], F16)
            wbT_sb = const.tile([P, EC, E], F16)
            ws_sb = const.tile([P, EC, E], F16)

            # written by every batch, consumed once at the end
            Scol16 = const.tile([P, EC, BPC], F16)
            out32 = const.tile([BPC, E], F32)
            nbias = const.tile([33, 1], F32)
            nc.gpsimd.memset(nbias[:], -8.0)

            def emit_loads(b):
                """Both orientations come straight from DRAM (host-transposed).
                Transposed tiles first: they feed S12, the head of each batch's
                PE chain; natural tiles aren't needed until S3/S4."""
                Tn = work.tile([P, LTC, E], F16, tag="Tn", bufs=3)
                In = work.tile([P, LVC, E], F16, tag="In", bufs=3)
                Ttr = work.tile([P, EC, LT], F16, tag="Ttr", bufs=3)
                Itr = work.tile([P, EC, LV], F16, tag="Itr", bufs=3)
                nc.sync.dma_start(Itr[:], imageT[b])
                if b == 0:
                    # wq rides between the two transposed loads: needed right
                    # after S2' (wvv), well before In/Tn
                    nc.sync.dma_start(wq_sb[:], wq_d)
                nc.sync.dma_start(Ttr[:], textT[b])
                nc.sync.dma_start(In[:, 0:4, :], image[b])
                nc.sync.dma_start(In[0:64, 4, :], image5[b])
                nc.sync.dma_start(Tn[:], text[b])
                return Tn, In, Ttr, Itr

            def emit_s12(b, Ttr, Itr):
                """wq_q [Lt, K] and wv_v [Lv, K] directly in natural layout:
                stationary = Ttr/Itr chunk [128e, 128pos], moving = w chunk."""
                wqqn = work.tile([P, LTC, K], F16, tag="wqqn", bufs=2)
                wvvn = work.tile([P, LVC, K], F16, tag="wvvn", bufs=2)
                # image side first (Itr is the first DMA to land), e-outer so
                # the accumulation consumes e-chunks as they stream in
                ps = psm.tile([P, 512], F32, tag="psm")
                for y in range(4):
                    for e in range(EC):
                        nc.tensor.matmul(
                            ps[:, 128 * y : 128 * (y + 1)],
                            Itr[:, e, 128 * y : 128 * (y + 1)],
                            wv_sb[:, e, :],
                            start=(e == 0),
                            stop=(e == EC - 1),
                        )
                nc.vector.tensor_copy(wvvn[:, 0:4, :], ps[:])
                ps = psm.tile([P, 512], F32, tag="psm")
                for e in range(EC):
                    nc.tensor.matmul(
                        ps[0:64, 0:128],
                        Itr[:, e, 512:576],
                        wv_sb[:, e, :],
                        start=(e == 0),
                        stop=(e == EC - 1),
                    )
                nc.scalar.activation(wvvn[0:64, 4, :], ps[0:64, 0:128], COPY)
                for h in range(2):  # x-chunks 0-3, 4-7
                    ps = psm.tile([P, 512], F32, tag="psm")
                    for j in range(4):
                        x = 4 * h + j
                        for e in range(EC):
                            nc.tensor.matmul(
                                ps[:, 128 * j : 128 * (j + 1)],
                                Ttr[:, e, 128 * x : 128 * (x + 1)],
                                wq_sb[:, e, :],
                                start=(e == 0),
                                stop=(e == EC - 1),
                            )
                    if h == 0:
                        nc.vector.tensor_copy(wqqn[:, 0:4, :], ps[:])
                    else:
                        nc.scalar.activation(wqqn[:, 4:8, :], ps[:], COPY)
                return wqqn, wvvn

            def emit_s3(b, Tn, wqqn):
                """A1 [E, K] natural: A1[e,k] = sum_x T[x,e] wq_q[x,k]."""
                A1n = work.tile([P, EC, K], F16, tag="A1n", bufs=1)
                pA = pnat.tile([P, EC * K], F32, tag="pnat")
                for e in range(EC):
                    for x in range(LTC):
                        nc.tensor.matmul(
                            pA[:, 128 * e : 128 * (e + 1)],
                            Tn[:, x, 128 * e : 128 * (e + 1)],
                            wqqn[:, x, :],
                            start=(x == 0),
                            stop=(x == LTC - 1),
                        )
                nc.scalar.activation(A1n[:], pA[:], COPY)
                return A1n

            def emit_s4(b, In, wvvn):
                B1n = work.tile([P, EC, K], F16, tag="B1n", bufs=1)
                pB = pnat.tile([P, EC * K], F32, tag="pnat")
                for e in range(EC):
                    for y in range(LVC):
                        pc = LV_CH[y]
                        nc.tensor.matmul(
                            pB[:, 128 * e : 128 * (e + 1)],
                            In[0:pc, y, 128 * e : 128 * (e + 1)],
                            wvvn[0:pc, y, :],
                            start=(y == 0),
                            stop=(y == LVC - 1),
                        )
                nc.scalar.activation(B1n[:], pB[:], COPY)
                return B1n

            def emit_s56(b, A1n, B1n):
                """A2 = w_b^T @ A1 + w_v;  B2 = w_b @ B1 + w_q (fold the h adds)."""
                B2n = work.tile([P, EC, K], F16, tag="B2n", bufs=1)
                pB = pnat.tile([P, EC * K], F32, tag="pnat")
                for i in range(EC):
                    for e in range(EC):
                        nc.tensor.matmul(
                            pB[:, 128 * i : 128 * (i + 1)],
                            wbT_sb[:, e, 128 * i : 128 * (i + 1)],
                            B1n[:, e, :],
                            start=(e == 0),
                            stop=(e == EC - 1),
                        )
                nc.vector.tensor_add(B2n[:], pB[:], wq_sb[:])
                A2n = work.tile([P, EC, K], F16, tag="A2n", bufs=1)
                pA = pnat.tile([P, EC * K], F32, tag="pnat")
                for i in range(EC):
                    for e in range(EC):
                        nc.tensor.matmul(
                            pA[:, 128 * i : 128 * (i + 1)],
                            wb_sb[:, e, 128 * i : 128 * (i + 1)],
                            A1n[:, e, :],
                            start=(e == 0),
                            stop=(e == EC - 1),
                        )
                nc.vector.tensor_add(A2n[:], pA[:], wv_sb[:])
                return A2n, B2n

            def emit_s78(b, Ttr, Itr, A2n, B2n, thunks=()):
                """h_vT = tanh(A2^T I^T) [K, Lv], h_qT = tanh(B2^T T^T) [K, Lt].

                The previous batch's tiny tail matmuls (thunks) are pumped in
                between these N=512 matmuls: a contiguous run of 1-col
                matmuls reads as idle to the PE activity monitor, which
                re-throttles the clock and makes the next real matmul burst
                start cold."""
                th = list(thunks)
                ti = 0
                npump = (len(th) + 23) // 24 if th else 0

                def pump():
                    nonlocal ti
                    for _ in range(npump):
                        if ti < len(th):
                            th[ti]()
                            ti += 1

                hqT = work.tile([P, LT], F16, tag="hqT", bufs=2)
                for h in range(2):
                    ps = psm.tile([P, 512], F32, tag="psm")
                    for e in range(EC):
                        nc.tensor.matmul(
                            ps[:],
                            B2n[:, e, :],
                            Ttr[:, e, 512 * h : 512 * (h + 1)],
                            start=(e == 0),
                            stop=(e == EC - 1),
                        )
                        pump()
                    nc.scalar.activation(
                        hqT[:, 512 * h : 512 * (h + 1)], ps[:], TANH
                    )
                hvT = work.tile([P, LV], F16, tag="hvT", bufs=2)
                for lo, hi in ((0, 512), (512, 576)):
                    ps = psm.tile([P, 512], F32, tag="psm")
                    for e in range(EC):
                        nc.tensor.matmul(
                            ps[:, 0 : hi - lo],
                            A2n[:, e, :],
                            Itr[:, e, lo:hi],
                            start=(e == 0),
                            stop=(e == EC - 1),
                        )
                        pump()
                    nc.scalar.activation(hvT[:, lo:hi], ps[:, 0 : hi - lo], TANH)
                while ti < len(th):
                    th[ti]()
                    ti += 1
                return hvT, hqT

            # ---- tail pieces for batch b (emitted during head of b+1) ----
            def tail_logits(b, hvT, hqT):
                """Batched logits+softmax: row 0 = v (576 cols), row 32 = q."""
                l32 = work.tile([33, LT], F32, tag="l32", bufs=1)
                # unused partitions/cols must hold a large negative so the
                # batched max/exp are unaffected
                nc.gpsimd.memset(l32[:], -30000.0)
                for lo, hi in ((0, 512), (512, 1024)):
                    ps = psm.tile([P, 512], F32, tag="psm")
                    nc.tensor.matmul(
                        ps[32:33, 0 : hi - lo], whvq_sb[:, 1:2], hqT[:, lo:hi],
                        start=True, stop=True,
                    )
                    nc.vector.tensor_copy(l32[32:33, lo:hi], ps[32:33, 0 : hi - lo])
                for lo, hi in ((0, 512), (512, 576)):
                    ps = psm.tile([P, 512], F32, tag="psm")
                    nc.tensor.matmul(
                        ps[0:1, 0 : hi - lo], whvq_sb[:, 0:1], hvT[:, lo:hi],
                        start=True, stop=True,
                    )
                    nc.vector.tensor_copy(l32[0:1, lo:hi], ps[0:1, 0 : hi - lo])
                # no max-subtraction needed: |logit| <= ||h||_2 ||w_h||_2
                # <= sqrt(128) * 0.577 = 6.6 (h is tanh-bounded), so a fixed
                # -8 bias makes exp() overflow-impossible and softmax is
                # shift-invariant. Saves a [33, 1024] DVE reduce per batch
                # and shortens the tail latency chain.
                e16 = work.tile([33, LT], F16, tag="e16", bufs=1)
                s32 = work.tile([33, 1], F32, tag="s32", bufs=1)
                # row-sum accumulates for free during the exp activation
                nc.scalar.activation(
                    e16[:], l32[:], EXP, bias=nbias[:], accum_out=s32[:]
                )
                r32 = work.tile([33, 1], F32, tag="r32", bufs=1)
                # rows 1-31 are padding with sum 0 -> 1/0 inf; only invert the
                # two live rows
                nc.vector.reciprocal(r32[0:1], s32[0:1])
                nc.vector.reciprocal(r32[32:33], s32[32:33])
                a32 = work.tile([33, LT], F32, tag="a32", bufs=1)
                nc.vector.tensor_scalar_mul(a32[0:1, :], e16[0:1, :], r32[0:1])
                nc.vector.tensor_scalar_mul(
                    a32[32:33, :], e16[32:33, :], r32[32:33]
                )
                return a32

            def build_tail_thunks(b, Tn, In, a32):
                """Tail tiny-matmul work as a thunk list: row->column
                transposes of av/aq, then the rank-1 context accumulation
                (cv+cq)^T [E, 1] directly in Scol column layout."""
                pt = pcx.tile([P, 32], F32, tag="pcx")
                avT = work.tile([P, LVC], F16, tag="avT", bufs=1)
                aqT = work.tile([P, LTC], F16, tag="aqT", bufs=1)
                thunks = []

                def tr_v(cy):
                    pc = LV_CH[cy]
                    nc.tensor.transpose(
                        pt[0:pc, cy : cy + 1],
                        a32[0:1, 128 * cy : 128 * cy + pc],
                        id32[0:1, 0:1],
                    )

                def cp_av():
                    nc.vector.tensor_copy(avT[:, 0:4], pt[:, 0:4])
                    nc.vector.tensor_copy(avT[0:64, 4:5], pt[0:64, 4:5])

                def tr_q(cx):
                    nc.tensor.transpose(
                        pt[:, 5 + cx : 6 + cx],
                        a32[32:33, 128 * cx : 128 * (cx + 1)],
                        id32[32:33, 32:33],
                    )

                def cp_aq():
                    nc.vector.tensor_copy(aqT[:], pt[:, 5:13])

                def mm_v(e, cy):
                    pc = LV_CH[cy]
                    nc.tensor.matmul(
                        pt[:, 16 + e : 17 + e],
                        In[0:pc, cy, 128 * e : 128 * (e + 1)],
                        avT[0:pc, cy : cy + 1],
                        start=(cy == 0),
                        stop=False,
                    )

                def mm_q(e, cx):
                    nc.tensor.matmul(
                        pt[:, 16 + e : 17 + e],
                        Tn[:, cx, 128 * e : 128 * (e + 1)],
                        aqT[:, cx : cx + 1],
                        start=False,
                        stop=(cx == LTC - 1),
                    )

                def cp_ctx(e):
                    nc.vector.tensor_copy(
                        Scol16[:, e, b : b + 1], pt[:, 16 + e : 17 + e]
                    )

                for cy in range(LVC):
                    thunks.append(lambda cy=cy: tr_v(cy))
                thunks.append(cp_av)
                for cx in range(LTC):
                    thunks.append(lambda cx=cx: tr_q(cx))
                thunks.append(cp_aq)
                for e in range(EC):
                    for cy in range(LVC):
                        thunks.append(lambda e=e, cy=cy: mm_v(e, cy))
                    for cx in range(LTC):
                        thunks.append(lambda e=e, cx=cx: mm_q(e, cx))
                    thunks.append(lambda e=e: cp_ctx(e))
                return thunks

            # ---- software-pipelined batch loop ----
            TnIn = {0: emit_loads(0)}
            # small logit weights queue behind batch 0's loads (first needed
            # at tail_logits(0), emitted during slot 1)
            nc.sync.dma_start(whvq_sb[:], whvq_d)
            state = {}
            for b in range(BPC):
                if b + 1 < BPC:
                    TnIn[b + 1] = emit_loads(b + 1)
                if b == 0:
                    nc.gpsimd.dma_start(wbT_sb[:], wbT_d)
                    nc.gpsimd.dma_start(wb_sb[:], wb_d)
                elif b == 2:
                    nc.gpsimd.dma_start(ws_sb[:], ws_d)
                Tn, In, Ttr, Itr = TnIn[b]
                wqqn, wvvn = emit_s12(b, Ttr, Itr)
                thunks = ()
                if b > 0:
                    pTn, pIn, phvT, phqT = state[b - 1]
                    a32 = tail_logits(b - 1, phvT, phqT)
                    thunks = build_tail_thunks(b - 1, pTn, pIn, a32)
                B1n = emit_s4(b, In, wvvn)
                A1n = emit_s3(b, Tn, wqqn)
                A2n, B2n = emit_s56(b, A1n, B1n)
                hvT, hqT = emit_s78(b, Ttr, Itr, A2n, B2n, thunks)
                state[b] = (Tn, In, hvT, hqT)
            # last batch's tail runs serial (nothing left to interleave with)
            Tn, In, hvT, hqT = state[BPC - 1]
            a32 = tail_logits(BPC - 1, hvT, hqT)
            for t in build_tail_thunks(BPC - 1, Tn, In, a32):
                t()

            # ---- out = tanh(S @ w_s) for all 8 batches ----
            # e-outer accumulation: the e-chunk matmuls only wait on that
            # chunk's Scol column copies, so this overlaps batch 7's tail
            psF0 = psm.tile([P, 512], F32, tag="psm")
            psF1 = psm.tile([P, 512], F32, tag="psm")
            for e in range(EC):
                nc.tensor.matmul(
                    psF0[0:BPC, 0:384], Scol16[:, e, :], ws_sb[:, e, 0:384],
                    start=(e == 0), stop=(e == EC - 1),
                )
                nc.tensor.matmul(
                    psF1[0:BPC, 0:384], Scol16[:, e, :], ws_sb[:, e, 384:768],
                    start=(e == 0), stop=(e == EC - 1),
                )
            nc.scalar.activation(out32[:, 0:384], psF0[0:BPC, 0:384], TANH)
            nc.scalar.activation(out32[:, 384:768], psF1[0:BPC, 0:384], TANH)
            nc.sync.dma_start(out_d[:], out32[:])

    if split_drains:
        _split_excess_waits(nc)
    return nc


_NC = None


def _get_nc():
    global _NC
    if _NC is None:
        _NC = build_nc()
    return _NC


def _make_in_maps(text, image, w_b, w_v, w_q, w_hv, w_hq, w_s):
    f16 = np.float16
    wb = np.asarray(w_b)

    def pack(w):
        # [C*P, inner] -> [P, C*inner] (SBUF layout, contiguous per partition)
        w = np.asarray(w, dtype=f16)
        r, n = w.shape
        return np.ascontiguousarray(
            w.reshape(r // P, P, n).transpose(1, 0, 2).reshape(P, -1)
        )

    weights = {
        "wq": pack(w_q),
        "wv": pack(w_v),
        "wb": pack(wb),
        "wbT": pack(wb.T),
        "whvq": np.ascontiguousarray(
            np.concatenate([np.asarray(w_hv), np.asarray(w_hq)], axis=1), dtype=f16
        ),
        "ws": pack(w_s),
    }
    text = np.asarray(text)
    image = np.asarray(image)
    in_maps = []
    for c in range(N_CORES):
        sl = slice(BPC * c, BPC * (c + 1))
        t16 = np.asarray(text[sl], dtype=f16)
        i16 = np.asarray(image[sl], dtype=f16)
        tT = t16.transpose(0, 2, 1)  # [b, E, LT]
        iT = i16.transpose(0, 2, 1)  # [b, E, LV]
        # SBUF layout: [b, p, c*inner]; row chunks fold as (c p), col chunks (c p) too
        def to_sbuf(x, inner):
            # x: [b, C*P, inner] -> [b, P, C*inner]
            b_, r, n = x.shape
            return np.ascontiguousarray(
                x.reshape(b_, r // P, P, n).transpose(0, 2, 1, 3).reshape(b_, P, -1)
            )
        in_maps.append(
            {
                "text": to_sbuf(t16, E),
                "image": to_sbuf(np.ascontiguousarray(i16[:, 0:512, :]), E),
                "image5": np.ascontiguousarray(i16[:, 512:576, :]),
                "textT": to_sbuf(tT, LT),
                "imageT": to_sbuf(iT, LV),
                **weights,
            }
        )
    return in_maps


def kernel(
    text_hidden_states,
    image_hidden_states,
    text_attention_mask,
    w_b,
    w_v,
    w_q,
    w_hv,
    w_hq,
    w_s,
    _trace=False,
):
    # text_attention_mask is all-ones and unused by the reference computation.
    in_maps = _make_in_maps(
        text_hidden_states, image_hidden_states, w_b, w_v, w_q, w_hv, w_hq, w_s
    )
    nc = _get_nc()
    res = bass_utils.run_bass_kernel_spmd(
        nc, in_maps, core_ids=list(range(N_CORES)), trace=_trace
    )
    out = np.concatenate([res.results[c]["out"] for c in range(N_CORES)], axis=0)
    if _trace:
        kernel._last_exec_time_ns = res.exec_time_ns
    return out.astype(np.float32)


kernel._last_exec_time_ns = None
